# revision 31
# baseline (speedup 1.0000x reference)
"""Trainium2 Bass kernel for dual cross-attention (CotSR block).

Problem: two cross-attentions between x1, x2 [B=4, C=512, H=W=64].
  q1 = wq1@x1, k2 = wk2@x2, v2 = wv2@x2 ; att1 = softmax(q1^T k2) over keys
  out1 = x1 + gamma1 * (v2 @ att1^T)   (and symmetrically for out2)

Sharding: 8 independent (batch, direction) jobs -> one per NeuronCore.
Each core runs the same SPMD program on its own [C, N] slices.

Per-core dataflow v2 (N = 4096 tokens, DQ = 64, C = 512):
  - Q/K projected with host-duplicated weights -> [128, N] bf16 (two copies
    of the 64-row result stacked), enabling row-packed (tile_position)
    concurrent S matmuls that contract over only dq=64.
  - VT = (wv@xkv)^T evicted straight to fp8e4 [128 keys, 32, 512 c].
  - Attention loop per 512-query block, per double-keytile dt (256 keys):
      ST[128k, 2, 512q] f32 <- two row-packed S matmuls (2 PSUM banks)
      PT = exp(ST - 7) -> fp8e4 [128, 2, 512]   (ONE wide ACT instruction)
      o[cc] += DoubleRow-matmul(VT[., 2dt:2dt+2, cc], PT)   (fp8, 256-contr)
      rs    += DoubleRow-matmul(invg_tile, PT)  = rowsum/gamma, broadcast
    exp(-7) scaling cancels between numerator and rowsum.
  - Tail: recip = 1/(rs/g) = g/rs (fast approx);  out = (o*recip + g*bv) + x
"""

import numpy as np
import ml_dtypes

import concourse.bass as bass
import concourse.mybir as mybir
import concourse.tile as tile
from concourse import bacc
import concourse.bass_utils as _bu

# walrus's --enable-ldw-opt=false serializes every LDWEIGHTS with its MATMUL
# (measured 379 ns/MM vs ~215 warm); enable background-weight-buffer overlap.
_orig_run_command = _bu.run_command


def _patched_run_command(argv, **kw):
    argv = ["--enable-ldw-opt=true" if a == "--enable-ldw-opt=false" else a
            for a in argv]
    return _orig_run_command(argv, **kw)


_bu.run_command = _patched_run_command
from concourse.bass_utils import run_bass_kernel_spmd
from concourse._compat import with_exitstack
from contextlib import ExitStack

F32 = mybir.dt.float32
F32R = mybir.dt.float32r
BF16 = mybir.dt.bfloat16
FP8 = mybir.dt.float8e4
AF = mybir.ActivationFunctionType
ALU = mybir.AluOpType
PM = mybir.MatmulPerfMode
ts = bass.ts

B, C, H, W = 4, 512, 64, 64
N = H * W          # 4096
DQ = 64
P = 128
QB = 512           # query block (free dim of ST / moving operand)
NQB = N // QB      # 8 query blocks
NKT = N // P       # 32 key tiles
NDT = NKT // 2     # 16 double key tiles (256 keys each, fp8 DoubleRow)
NCC = C // P       # 4 channel chunks
SHIFT = 7.0        # exp(S - SHIFT): keeps fp8e4 in range; cancels in softmax


@with_exitstack
def _body(ctx: ExitStack, tc: "tile.TileContext", io: dict):
    nc = tc.nc
    xq_d, xkv_d, wq_d, wk_d, wv_d = io["xq"], io["xkv"], io["wq"], io["wk"], io["wv"]
    bq_d, bk_d = io["bq"], io["bk"]
    gbv_d, invg_d, out_d = io["gbv"], io["invg"], io["out"]

    const = ctx.enter_context(tc.tile_pool(name="const", bufs=1))
    persist = ctx.enter_context(tc.tile_pool(name="persist", bufs=1))
    wpool = ctx.enter_context(tc.tile_pool(name="wpool", bufs=1))
    stage = ctx.enter_context(tc.tile_pool(name="stage", bufs=3))
    ptp = ctx.enter_context(tc.tile_pool(name="ptp", bufs=6))
    dvp = ctx.enter_context(tc.tile_pool(name="dvp", bufs=3))
    pso = ctx.enter_context(tc.tile_pool(name="pso", bufs=1, space="PSUM"))
    pst = ctx.enter_context(tc.tile_pool(name="pst", bufs=1, space="PSUM"))
    psr = ctx.enter_context(tc.tile_pool(name="psr", bufs=2, space="PSUM"))

    # ---- small inputs ----
    bq_sb = const.tile([P, 1], F32, tag="bq", name="bq_sb")
    nc.sync.dma_start(bq_sb, bq_d)
    bk_sb = const.tile([P, 1], F32, tag="bk", name="bk_sb")
    nc.sync.dma_start(bk_sb, bk_d)
    gbv_sb = const.tile([P, NCC], F32, tag="gbv", name="gbv_sb")
    nc.sync.dma_start(gbv_sb, gbv_d)
    invg_st = const.tile([P, 2 * P], F32, tag="invg_st", name="invg_st")
    nc.sync.dma_start(invg_st, invg_d)
    invg8 = const.tile([P, 2, P], FP8, tag="invg8", name="invg8")
    nc.vector.tensor_copy(invg8[:, 0, :], invg_st[:, 0:P])
    nc.vector.tensor_copy(invg8[:, 1, :], invg_st[:, P:2 * P])
    nshift = const.tile([P, 1], F32, tag="nshift", name="nshift")
    nc.vector.memset(nshift, -SHIFT)

    # ---- weights arrive PRE-TRANSPOSED from host (see make_in_maps):
    # wq_d [C, 128] (= [wq.T | wq.T]/8), wk8_d [C, 128] (= dup(wk.T)*8),
    # wv_d [C, C] = wv.T*8.  wk/wv go to fp8 for DoubleRow projections. ----
    wqT = wpool.tile([P, NCC * P], F32R, tag="wqT", name="wqT")
    wk8 = wpool.tile([P, NCC, P], FP8, tag="wk8", name="wk8")
    wvT8 = wpool.tile([P, NCC, C], FP8, tag="wvT8", name="wvT8")
    for j in range(NCC):
        nc.sync.dma_start(wqT[:, ts(j, P)], wq_d[ts(j, P), :])
        wstk = stage.tile([P, P], F32, tag="w_stagek", name="w_stk")
        nc.sync.dma_start(wstk, wk_d[ts(j, P), :])
        nc.vector.tensor_copy(wk8[:, j, :], wstk)
        wst2 = stage.tile([P, C], F32, tag="w_stage2", name="w_st2")
        nc.sync.dma_start(wst2, wv_d[ts(j, P), :])
        nc.vector.tensor_copy(wvT8[:, j, :], wst2)

    # ---- xq resident f32 via the fast HWDGE (no cast: fp32r matmul reads
    # it directly, as does the residual); xkv fp8 via SWDGE cast (its only
    # user now, so the packet-rate-bound cast path carries half the bytes).
    # The two queues run in parallel.
    xq_f = [persist.tile([P, N], F32R, tag=f"xq{cc}", name=f"xq_f{cc}")
            for cc in range(NCC)]
    xkv8 = persist.tile([P, NCC, N], FP8, tag="xkv8", name="xkv8")
    for cc in range(NCC):  # xq piece 0 only (Q0); pieces 1-3 are issued
        nc.sync.dma_start(xq_f[cc][:, ts(0, 1024)],  # from inside qb0 so
                          xq_d[ts(cc, P), ts(0, 1024)])  # xkv gets the HBM
    for h in range(4):
        for cc in range(NCC):
            nc.gpsimd.dma_start(xkv8[:, cc, ts(h, 1024)],
                                xkv_d[ts(cc, P), ts(h, 1024)])

    # ---- projections (K/V DoubleRow fp8, emitted per DMA piece; Q lazy) ----
    # Q/K [128, N] bf16 with duplicated halves (partitions 0-63 == 64-127)
    Q_sb = persist.tile([P, N], BF16, tag="Q", name="Q_sb")
    K_sb = persist.tile([P, N], BF16, tag="K", name="K_sb")
    VT_sb = persist.tile([P, NKT, C], FP8, tag="VT", name="VT_sb")

    def q_proj(nb, q_ps):
        for cc in range(NCC):
            nc.tensor.matmul(q_ps, wqT[:, ts(cc, P)],
                             xq_f[cc][:, ts(nb, QB)],
                             start=(cc == 0), stop=(cc == NCC - 1))
        nc.vector.tensor_scalar_add(Q_sb[:, ts(nb, QB)], q_ps, bq_sb)

    otags = [f"o{i}" for i in range(NCC)]
    # Q0 first: xq piece 0 lands on the fast queue well before xkv piece 0
    q_proj(0, pso.tile([P, QB], F32, tag=otags[0], name="q_ps"))
    for h in range(4):
        for nb in (2 * h, 2 * h + 1):
            kp = pso.tile([P, QB], F32, tag=otags[3], name="k_ps")
            for dc in range(2):
                nc.tensor.matmul(kp, wk8[:, 2 * dc:2 * dc + 2, :],
                                 xkv8[:, 2 * dc:2 * dc + 2, ts(nb, QB)],
                                 start=(dc == 0), stop=(dc == 1),
                                 perf_mode=PM.DoubleRow)
            nc.vector.tensor_scalar_add(K_sb[:, ts(nb, QB)], kp, bk_sb)
        for nt in range(8 * h, 8 * h + 8):
            vp = pso.tile([P, C], F32, tag=otags[nt % 3], name="v_ps")
            for dc in range(2):
                nc.tensor.matmul(vp, xkv8[:, 2 * dc:2 * dc + 2, ts(nt, P)],
                                 wvT8[:, 2 * dc:2 * dc + 2, :],
                                 start=(dc == 0), stop=(dc == 1),
                                 perf_mode=PM.DoubleRow)
            nc.vector.tensor_copy(VT_sb[:, nt, :], vp)

    # ---- attention main loop ----
    def tail_pre(qb, o_ps, rs_ps):
        # free o banks ASAP; all on DVE so the ACT queue stays a pure
        # exp stream (an ACT copy here would stall the next qblock's exps)
        recip = dvp.tile([P, QB], F32, tag="recip", name="recip", bufs=2)
        nc.vector.reciprocal_approx_fast(recip, rs_ps)
        o_sb = []
        for cc in range(NCC):
            osb = dvp.tile([P, QB], F32, tag=f"osb{cc % 2}", name="o_sb", bufs=4)
            nc.vector.tensor_copy(osb, o_ps[cc])
            o_sb.append(osb)
        return qb, o_sb, recip

    def tail_post(qb, o_sb, recip):
        for cc in range(NCC):
            t1 = dvp.tile([P, QB], F32, tag="t1", name="t1")
            nc.vector.tensor_mul(t1, o_sb[cc], recip)
            og = dvp.tile([P, QB], F32, tag="og", name="og")
            nc.vector.scalar_tensor_tensor(og, t1, gbv_sb[:, cc:cc + 1],
                                           xq_f[cc][:, ts(qb, QB)].bitcast(F32),
                                           op0=ALU.add, op1=ALU.add)
            nc.sync.dma_start(out_d[ts(cc, P), ts(qb, QB)], og)

    def pv_emit(o_ps, rs_ps, pt, dt):
        # PV + rowsum for double-tile dt; emitted a few S/exp slots later so
        # these matmuls overlap ACT's exp stream on the PE queue (and qb-1's
        # last PVs fill the next qblock's pipeline-refill bubble).
        for cc in range(NCC):
            nc.tensor.matmul(o_ps[cc], VT_sb[:, 2 * dt:2 * dt + 2, ts(cc, P)],
                             pt, start=(dt == 0), stop=(dt == NDT - 1),
                             perf_mode=PM.DoubleRow)
        nc.tensor.matmul(rs_ps, invg8, pt,
                         start=(dt == 0), stop=(dt == NDT - 1),
                         perf_mode=PM.DoubleRow)

    PEND = 3
    pend = []  # global across qblocks: (o_ps, rs_ps, pt, dt)
    prev = None
    for qb in range(NQB):
        o_ps = [pso.tile([P, QB], F32, tag=otags[cc], name=f"o_ps{cc}")
                for cc in range(NCC)]
        rs_ps = psr.tile([P, QB], F32, tag="rs", name="rs_ps")
        pre = None
        for dt in range(NDT):
            st = pst.tile([P, 2, QB], F32, tag="st", name="st_ps")
            nc.tensor.matmul(st[:, 0, :], K_sb[0:DQ, ts(2 * dt, P)],
                             Q_sb[0:DQ, ts(qb, QB)],
                             start=True, stop=True, tile_position=(0, 0))
            nc.tensor.matmul(st[:, 1, :], K_sb[DQ:P, ts(2 * dt + 1, P)],
                             Q_sb[DQ:P, ts(qb, QB)],
                             start=True, stop=True, tile_position=(DQ, 0))
            pt = ptp.tile([P, 2, QB], FP8, tag="pt", name="pt_sb")
            nc.scalar.activation(pt, st, AF.Exp, bias=nshift)
            pend.append((o_ps, rs_ps, pt, dt))
            if len(pend) > PEND:
                pv_emit(*pend.pop(0))
            if dt == 2 and prev is not None:
                pre = tail_pre(*prev)
            if dt == 4 and pre is not None:
                tail_post(*pre)
            if qb == 0 and dt in (0, 2, 4):
                # deferred xq pieces 1-3: issued off the ACT queue (HWDGE)
                # so the ramp's HBM read bandwidth goes to xkv first
                for cc in range(NCC):
                    nc.scalar.dma_start(xq_f[cc][:, ts(dt // 2 + 1, 1024)],
                                        xq_d[ts(cc, P), ts(dt // 2 + 1, 1024)])
            if 10 <= dt <= 13 and qb < NQB - 1:
                # spare psr slot; one chunk per dt fits the per-dt PE slack
                cc = dt - 10
                if cc == 0:
                    qp_ps = psr.tile([P, QB], F32, tag="rs", name="q_ps")
                nc.tensor.matmul(qp_ps, wqT[:, ts(cc, P)],
                                 xq_f[cc][:, ts(qb + 1, QB)],
                                 start=(cc == 0), stop=(cc == NCC - 1))
                if cc == 3:
                    nc.vector.tensor_scalar_add(Q_sb[:, ts(qb + 1, QB)],
                                                qp_ps, bq_sb)
        prev = (qb, o_ps, rs_ps)
    for e in pend:
        pv_emit(*e)
    tail_post(*tail_pre(*prev))


_NC_CACHE = {}


def _fuse_ldweights(nc):
    """Re-fuse Tile's split LDWEIGHTS+MATMUL pairs into self-loading matmuls
    so walrus's ldw-opt (background weight buffer) can overlap weight loads
    with in-flight matmuls. Any pending LDWEIGHTS sync-waits are merged into
    the next MATMUL."""
    for b in nc.m.functions[0].blocks:
        out = []
        pending = []
        for i in b.instructions:
            tn = type(i).__name__
            if tn == "InstLdweights":
                pending.append(i)
                continue
            if tn == "InstMatmult" and pending:
                i.ldweights = True
                for p in pending:
                    si = p.sync_info
                    if si is not None and (si.on_wait or si.on_update):
                        if i.sync_info is None:
                            i.sync_info = mybir.SyncInfo(on_wait=[], on_update=[])
                        i.sync_info.on_wait = (list(si.on_wait)
                                               + list(i.sync_info.on_wait))
                        i.sync_info.on_update = (list(si.on_update)
                                                 + list(i.sync_info.on_update))
                pending = []
            out.append(i)
        assert not pending, "trailing ldweights without matmul"
        b.instructions[:] = out


def _build():
    if "nc" in _NC_CACHE:
        return _NC_CACHE["nc"]
    nc = bacc.Bacc("TRN2", target_bir_lowering=False, debug=False, num_devices=8)
    io = {
        "xq": nc.dram_tensor("xq", [C, N], F32R, kind="ExternalInput").ap(),
        "xkv": nc.dram_tensor("xkv", [C, N], F32, kind="ExternalInput").ap(),
        "wq": nc.dram_tensor("wq", [C, P], F32R, kind="ExternalInput").ap(),
        "wk": nc.dram_tensor("wk", [C, P], F32, kind="ExternalInput").ap(),
        "wv": nc.dram_tensor("wv", [C, C], F32, kind="ExternalInput").ap(),
        "bq": nc.dram_tensor("bq", [P, 1], F32, kind="ExternalInput").ap(),
        "bk": nc.dram_tensor("bk", [P, 1], F32, kind="ExternalInput").ap(),
        "gbv": nc.dram_tensor("gbv", [P, NCC], F32, kind="ExternalInput").ap(),
        "invg": nc.dram_tensor("invg", [P, 2 * P], F32, kind="ExternalInput").ap(),
        "out": nc.dram_tensor("out", [C, N], F32, kind="ExternalOutput").ap(),
    }
    with tile.TileContext(nc) as tc:
        _body(tc, io)
    _fuse_ldweights(nc)
    nc.compile()
    _NC_CACHE["nc"] = nc
    return nc


def make_in_maps(x1, x2, wq1, bq1, wk1, bk1, wv1, bv1,
                 wq2, bq2, wk2, bk2, wv2, bv2, gamma1, gamma2):
    """Returns the 8 per-core input dicts. Cores 0-3: out1[b]; 4-7: out2[b]."""
    f = np.ascontiguousarray
    x1f = np.asarray(x1, np.float32).reshape(B, C, N)
    x2f = np.asarray(x2, np.float32).reshape(B, C, N)

    # wk/wv scaled x8 into fp8's normal range; compensated by wq,bq /8 and
    # invg = 8/g (the recip path divides the x8-scaled o by rs*8/g).
    WS = 8.0

    def dup_t(w, s):  # [DQ, C] -> [C, 128] duplicated transpose, scaled
        wt = np.asarray(w, np.float32).T * s
        return f(np.concatenate([wt, wt], axis=1))

    def dup_b(b, s):  # [DQ] -> [128, 1]
        bb = np.asarray(b, np.float32).reshape(DQ) * s
        return f(np.concatenate([bb, bb]).reshape(P, 1))

    def common(wq, bq, wk, bk, wv, bv, gamma):
        g = float(np.asarray(gamma).reshape(-1)[0])
        gbv = (g * np.asarray(bv, np.float32)).reshape(NCC, P).T
        return {
            "wq": dup_t(wq, 1.0 / WS), "wk": dup_t(wk, WS),
            "wv": f(np.asarray(wv, np.float32).T * WS),
            "bq": dup_b(bq, 1.0 / WS), "bk": dup_b(bk, WS),
            "gbv": f(gbv),
            "invg": f(np.full((P, 2 * P), WS / g, np.float32)),
        }

    c1 = common(wq1, bq1, wk2, bk2, wv2, bv2, gamma1)
    c2 = common(wq2, bq2, wk1, bk1, wv1, bv1, gamma2)
    maps = []
    for b in range(B):
        maps.append({"xq": f(x1f[b]), "xkv": f(x2f[b]), **c1})
    for b in range(B):
        maps.append({"xq": f(x2f[b]), "xkv": f(x1f[b]), **c2})
    return maps


def kernel(**inputs):
    nc = _build()
    in_maps = make_in_maps(**inputs)
    res = run_bass_kernel_spmd(nc, in_maps, list(range(8))).results
    out1 = np.stack([res[b]["out"].reshape(C, H, W) for b in range(B)])
    out2 = np.stack([res[B + b]["out"].reshape(C, H, W) for b in range(B)])
    return out1, out2


# revision 33
# speedup vs baseline: 1.0735x; 1.0735x over previous
"""Trainium2 Bass kernel for dual cross-attention (CotSR block).

Problem: two cross-attentions between x1, x2 [B=4, C=512, H=W=64].
  q1 = wq1@x1, k2 = wk2@x2, v2 = wv2@x2 ; att1 = softmax(q1^T k2) over keys
  out1 = x1 + gamma1 * (v2 @ att1^T)   (and symmetrically for out2)

Sharding: 8 independent (batch, direction) jobs -> one per NeuronCore.
Each core runs the same SPMD program on its own [C, N] slices.

Per-core dataflow v2 (N = 4096 tokens, DQ = 64, C = 512):
  - Q/K projected with host-duplicated weights -> [128, N] bf16 (two copies
    of the 64-row result stacked), enabling row-packed (tile_position)
    concurrent S matmuls that contract over only dq=64.
  - VT = (wv@xkv)^T evicted straight to fp8e4 [128 keys, 32, 512 c].
  - Attention loop per 512-query block, per double-keytile dt (256 keys):
      ST[128k, 2, 512q] f32 <- two row-packed S matmuls (2 PSUM banks)
      PT = exp(ST - 7) -> fp8e4 [128, 2, 512]   (ONE wide ACT instruction)
      o[cc] += DoubleRow-matmul(VT[., 2dt:2dt+2, cc], PT)   (fp8, 256-contr)
      rs    += DoubleRow-matmul(invg_tile, PT)  = rowsum/gamma, broadcast
    exp(-7) scaling cancels between numerator and rowsum.
  - Tail: recip = 1/(rs/g) = g/rs (fast approx);  out = (o*recip + g*bv) + x
"""

import numpy as np
import ml_dtypes

import concourse.bass as bass
import concourse.mybir as mybir
import concourse.tile as tile
from concourse import bacc
import concourse.bass_utils as _bu

# walrus's --enable-ldw-opt=false serializes every LDWEIGHTS with its MATMUL
# (measured 379 ns/MM vs ~215 warm); enable background-weight-buffer overlap.
_orig_run_command = _bu.run_command


def _patched_run_command(argv, **kw):
    argv = ["--enable-ldw-opt=true" if a == "--enable-ldw-opt=false" else a
            for a in argv]
    return _orig_run_command(argv, **kw)


_bu.run_command = _patched_run_command
from concourse.bass_utils import run_bass_kernel_spmd
from concourse._compat import with_exitstack
from contextlib import ExitStack

F32 = mybir.dt.float32
F32R = mybir.dt.float32r
BF16 = mybir.dt.bfloat16
FP8 = mybir.dt.float8e4
AF = mybir.ActivationFunctionType
ALU = mybir.AluOpType
PM = mybir.MatmulPerfMode
ts = bass.ts

B, C, H, W = 4, 512, 64, 64
N = H * W          # 4096
DQ = 64
P = 128
QB = 512           # query block (free dim of ST / moving operand)
NQB = N // QB      # 8 query blocks
NKT = N // P       # 32 key tiles
NDT = NKT // 2     # 16 double key tiles (256 keys each, fp8 DoubleRow)
NCC = C // P       # 4 channel chunks
SHIFT = 7.0        # exp(S - SHIFT): keeps fp8e4 in range; cancels in softmax


@with_exitstack
def _body(ctx: ExitStack, tc: "tile.TileContext", io: dict):
    nc = tc.nc
    xq_d, xkv_d, wq_d, wk_d, wv_d = io["xq"], io["xkv"], io["wq"], io["wk"], io["wv"]
    bq_d, bk_d = io["bq"], io["bk"]
    gbv_d, invg_d, out_d = io["gbv"], io["invg"], io["out"]

    const = ctx.enter_context(tc.tile_pool(name="const", bufs=1))
    persist = ctx.enter_context(tc.tile_pool(name="persist", bufs=1))
    wpool = ctx.enter_context(tc.tile_pool(name="wpool", bufs=1))
    stage = ctx.enter_context(tc.tile_pool(name="stage", bufs=3))
    ptp = ctx.enter_context(tc.tile_pool(name="ptp", bufs=6))
    dvp = ctx.enter_context(tc.tile_pool(name="dvp", bufs=3))
    pso = ctx.enter_context(tc.tile_pool(name="pso", bufs=1, space="PSUM"))
    pst = ctx.enter_context(tc.tile_pool(name="pst", bufs=1, space="PSUM"))
    psr = ctx.enter_context(tc.tile_pool(name="psr", bufs=2, space="PSUM"))

    # ---- small inputs ----
    bq_sb = const.tile([P, 1], F32, tag="bq", name="bq_sb")
    nc.sync.dma_start(bq_sb, bq_d)
    bk_sb = const.tile([P, 1], F32, tag="bk", name="bk_sb")
    nc.sync.dma_start(bk_sb, bk_d)
    gbv_sb = const.tile([P, NCC], F32, tag="gbv", name="gbv_sb")
    nc.sync.dma_start(gbv_sb, gbv_d)
    invg_st = const.tile([P, 2 * P], F32, tag="invg_st", name="invg_st")
    nc.sync.dma_start(invg_st, invg_d)
    invg8 = const.tile([P, 2, P], FP8, tag="invg8", name="invg8")
    nc.vector.tensor_copy(invg8[:, 0, :], invg_st[:, 0:P])
    nc.vector.tensor_copy(invg8[:, 1, :], invg_st[:, P:2 * P])
    nshift = const.tile([P, 1], F32, tag="nshift", name="nshift")
    nc.vector.memset(nshift, -SHIFT)

    # ---- weights arrive PRE-TRANSPOSED from host (see make_in_maps):
    # wq_d [C, 128] (= [wq.T | wq.T]/8), wk8_d [C, 128] (= dup(wk.T)*8),
    # wv_d [C, C] = wv.T*8.  wk/wv go to fp8 for DoubleRow projections. ----
    wqT = wpool.tile([P, NCC * P], F32R, tag="wqT", name="wqT")
    wk8 = wpool.tile([P, NCC, P], FP8, tag="wk8", name="wk8")
    wvT8 = wpool.tile([P, NCC, C], FP8, tag="wvT8", name="wvT8")
    for j in range(NCC):
        nc.sync.dma_start(wqT[:, ts(j, P)], wq_d[ts(j, P), :])
        wstk = stage.tile([P, P], F32, tag="w_stagek", name="w_stk")
        nc.sync.dma_start(wstk, wk_d[ts(j, P), :])
        nc.vector.tensor_copy(wk8[:, j, :], wstk)
        wst2 = stage.tile([P, C], F32, tag="w_stage2", name="w_st2")
        nc.sync.dma_start(wst2, wv_d[ts(j, P), :])
        nc.vector.tensor_copy(wvT8[:, j, :], wst2)

    # ---- xq resident f32 via the fast HWDGE (no cast: fp32r matmul reads
    # it directly, as does the residual); xkv fp8 via SWDGE cast (its only
    # user now, so the packet-rate-bound cast path carries half the bytes).
    # The two queues run in parallel.
    xq_f = [persist.tile([P, N], F32R, tag=f"xq{cc}", name=f"xq_f{cc}")
            for cc in range(NCC)]
    xkv8 = persist.tile([P, NCC, N], FP8, tag="xkv8", name="xkv8")
    for h in range(4):  # 1024-col pieces
        for cc in range(NCC):
            nc.sync.dma_start(xq_f[cc][:, ts(h, 1024)],
                              xq_d[ts(cc, P), ts(h, 1024)])
    for h in range(4):
        for cc in range(NCC):
            nc.gpsimd.dma_start(xkv8[:, cc, ts(h, 1024)],
                                xkv_d[ts(cc, P), ts(h, 1024)])

    # ---- projections (K/V DoubleRow fp8, emitted per DMA piece; Q lazy) ----
    # Q/K [128, N] bf16 with duplicated halves (partitions 0-63 == 64-127)
    Q_sb = persist.tile([P, N], BF16, tag="Q", name="Q_sb")
    K_sb = persist.tile([P, N], BF16, tag="K", name="K_sb")
    VT_sb = persist.tile([P, NKT, C], FP8, tag="VT", name="VT_sb")

    def q_proj(nb, q_ps):
        for cc in range(NCC):
            nc.tensor.matmul(q_ps, wqT[:, ts(cc, P)],
                             xq_f[cc][:, ts(nb, QB)],
                             start=(cc == 0), stop=(cc == NCC - 1))
        nc.vector.tensor_scalar_add(Q_sb[:, ts(nb, QB)], q_ps, bq_sb)

    otags = [f"o{i}" for i in range(NCC)]
    # Q0 first: xq piece 0 lands on the fast queue well before xkv piece 0
    q_proj(0, pso.tile([P, QB], F32, tag=otags[0], name="q_ps"))
    for h in range(4):
        for nb in (2 * h, 2 * h + 1):
            kp = pso.tile([P, QB], F32, tag=otags[3], name="k_ps")
            for dc in range(2):
                nc.tensor.matmul(kp, wk8[:, 2 * dc:2 * dc + 2, :],
                                 xkv8[:, 2 * dc:2 * dc + 2, ts(nb, QB)],
                                 start=(dc == 0), stop=(dc == 1),
                                 perf_mode=PM.DoubleRow)
            nc.vector.tensor_scalar_add(K_sb[:, ts(nb, QB)], kp, bk_sb)
        for nt in range(8 * h, 8 * h + 8):
            vp = pso.tile([P, C], F32, tag=otags[nt % 3], name="v_ps")
            for dc in range(2):
                nc.tensor.matmul(vp, xkv8[:, 2 * dc:2 * dc + 2, ts(nt, P)],
                                 wvT8[:, 2 * dc:2 * dc + 2, :],
                                 start=(dc == 0), stop=(dc == 1),
                                 perf_mode=PM.DoubleRow)
            nc.vector.tensor_copy(VT_sb[:, nt, :], vp)

    # ---- attention main loop ----
    def tail_pre(qb, o_ps, rs_ps):
        # free o banks ASAP; all on DVE so the ACT queue stays a pure
        # exp stream (an ACT copy here would stall the next qblock's exps)
        recip = dvp.tile([P, QB], F32, tag="recip", name="recip", bufs=2)
        nc.vector.reciprocal_approx_fast(recip, rs_ps)
        o_sb = []
        for cc in range(NCC):
            osb = dvp.tile([P, QB], F32, tag=f"osb{cc % 2}", name="o_sb", bufs=4)
            nc.vector.tensor_copy(osb, o_ps[cc])
            o_sb.append(osb)
        return qb, o_sb, recip

    def tail_post(qb, o_sb, recip):
        for cc in range(NCC):
            t1 = dvp.tile([P, QB], F32, tag="t1", name="t1")
            nc.vector.tensor_mul(t1, o_sb[cc], recip)
            og = dvp.tile([P, QB], F32, tag="og", name="og")
            nc.vector.scalar_tensor_tensor(og, t1, gbv_sb[:, cc:cc + 1],
                                           xq_f[cc][:, ts(qb, QB)].bitcast(F32),
                                           op0=ALU.add, op1=ALU.add)
            nc.sync.dma_start(out_d[ts(cc, P), ts(qb, QB)], og)

    def pv_emit(o_ps, rs_ps, pt, dt):
        # PV + rowsum for double-tile dt; emitted a few S/exp slots later so
        # these matmuls overlap ACT's exp stream on the PE queue (and qb-1's
        # last PVs fill the next qblock's pipeline-refill bubble).
        for cc in range(NCC):
            nc.tensor.matmul(o_ps[cc], VT_sb[:, 2 * dt:2 * dt + 2, ts(cc, P)],
                             pt, start=(dt == 0), stop=(dt == NDT - 1),
                             perf_mode=PM.DoubleRow)
        nc.tensor.matmul(rs_ps, invg8, pt,
                         start=(dt == 0), stop=(dt == NDT - 1),
                         perf_mode=PM.DoubleRow)

    PEND = 3
    pend = []  # global across qblocks: (o_ps, rs_ps, pt, dt)
    prev = None
    for qb in range(NQB):
        o_ps = [pso.tile([P, QB], F32, tag=otags[cc], name=f"o_ps{cc}")
                for cc in range(NCC)]
        rs_ps = psr.tile([P, QB], F32, tag="rs", name="rs_ps")
        pre = None
        for dt in range(NDT):
            st = pst.tile([P, 2, QB], F32, tag="st", name="st_ps")
            nc.tensor.matmul(st[:, 0, :], K_sb[0:DQ, ts(2 * dt, P)],
                             Q_sb[0:DQ, ts(qb, QB)],
                             start=True, stop=True, tile_position=(0, 0))
            nc.tensor.matmul(st[:, 1, :], K_sb[DQ:P, ts(2 * dt + 1, P)],
                             Q_sb[DQ:P, ts(qb, QB)],
                             start=True, stop=True, tile_position=(DQ, 0))
            pt = ptp.tile([P, 2, QB], FP8, tag="pt", name="pt_sb")
            nc.scalar.activation(pt, st, AF.Exp, bias=nshift)
            pend.append((o_ps, rs_ps, pt, dt))
            if len(pend) > PEND:
                pv_emit(*pend.pop(0))
            if dt == 2 and prev is not None:
                pre = tail_pre(*prev)
            if dt == 4 and pre is not None:
                tail_post(*pre)
            if 10 <= dt <= 13 and qb < NQB - 1:
                # spare psr slot; one chunk per dt fits the per-dt PE slack
                cc = dt - 10
                if cc == 0:
                    qp_ps = psr.tile([P, QB], F32, tag="rs", name="q_ps")
                nc.tensor.matmul(qp_ps, wqT[:, ts(cc, P)],
                                 xq_f[cc][:, ts(qb + 1, QB)],
                                 start=(cc == 0), stop=(cc == NCC - 1))
                if cc == 3:
                    nc.vector.tensor_scalar_add(Q_sb[:, ts(qb + 1, QB)],
                                                qp_ps, bq_sb)
        prev = (qb, o_ps, rs_ps)
    for e in pend:
        pv_emit(*e)
    tail_post(*tail_pre(*prev))


_NC_CACHE = {}


def _fuse_ldweights(nc):
    """Re-fuse Tile's split LDWEIGHTS+MATMUL pairs into self-loading matmuls
    so walrus's ldw-opt (background weight buffer) can overlap weight loads
    with in-flight matmuls. Any pending LDWEIGHTS sync-waits are merged into
    the next MATMUL."""
    for b in nc.m.functions[0].blocks:
        out = []
        pending = []
        for i in b.instructions:
            tn = type(i).__name__
            if tn == "InstLdweights":
                pending.append(i)
                continue
            if tn == "InstMatmult" and pending:
                i.ldweights = True
                for p in pending:
                    si = p.sync_info
                    if si is not None and (si.on_wait or si.on_update):
                        if i.sync_info is None:
                            i.sync_info = mybir.SyncInfo(on_wait=[], on_update=[])
                        i.sync_info.on_wait = (list(si.on_wait)
                                               + list(i.sync_info.on_wait))
                        i.sync_info.on_update = (list(si.on_update)
                                                 + list(i.sync_info.on_update))
                pending = []
            out.append(i)
        assert not pending, "trailing ldweights without matmul"
        b.instructions[:] = out


def _build():
    if "nc" in _NC_CACHE:
        return _NC_CACHE["nc"]
    nc = bacc.Bacc("TRN2", target_bir_lowering=False, debug=False, num_devices=8)
    io = {
        "xq": nc.dram_tensor("xq", [C, N], F32R, kind="ExternalInput").ap(),
        "xkv": nc.dram_tensor("xkv", [C, N], F32, kind="ExternalInput").ap(),
        "wq": nc.dram_tensor("wq", [C, P], F32R, kind="ExternalInput").ap(),
        "wk": nc.dram_tensor("wk", [C, P], F32, kind="ExternalInput").ap(),
        "wv": nc.dram_tensor("wv", [C, C], F32, kind="ExternalInput").ap(),
        "bq": nc.dram_tensor("bq", [P, 1], F32, kind="ExternalInput").ap(),
        "bk": nc.dram_tensor("bk", [P, 1], F32, kind="ExternalInput").ap(),
        "gbv": nc.dram_tensor("gbv", [P, NCC], F32, kind="ExternalInput").ap(),
        "invg": nc.dram_tensor("invg", [P, 2 * P], F32, kind="ExternalInput").ap(),
        "out": nc.dram_tensor("out", [C, N], F32, kind="ExternalOutput").ap(),
    }
    with tile.TileContext(nc) as tc:
        _body(tc, io)
    _fuse_ldweights(nc)
    nc.compile()
    _NC_CACHE["nc"] = nc
    return nc


def make_in_maps(x1, x2, wq1, bq1, wk1, bk1, wv1, bv1,
                 wq2, bq2, wk2, bk2, wv2, bv2, gamma1, gamma2):
    """Returns the 8 per-core input dicts. Cores 0-3: out1[b]; 4-7: out2[b]."""
    f = np.ascontiguousarray
    x1f = np.asarray(x1, np.float32).reshape(B, C, N)
    x2f = np.asarray(x2, np.float32).reshape(B, C, N)

    # wk/wv scaled x8 into fp8's normal range; compensated by wq,bq /8 and
    # invg = 8/g (the recip path divides the x8-scaled o by rs*8/g).
    WS = 8.0

    def dup_t(w, s):  # [DQ, C] -> [C, 128] duplicated transpose, scaled
        wt = np.asarray(w, np.float32).T * s
        return f(np.concatenate([wt, wt], axis=1))

    def dup_b(b, s):  # [DQ] -> [128, 1]
        bb = np.asarray(b, np.float32).reshape(DQ) * s
        return f(np.concatenate([bb, bb]).reshape(P, 1))

    def common(wq, bq, wk, bk, wv, bv, gamma):
        g = float(np.asarray(gamma).reshape(-1)[0])
        gbv = (g * np.asarray(bv, np.float32)).reshape(NCC, P).T
        return {
            "wq": dup_t(wq, 1.0 / WS), "wk": dup_t(wk, WS),
            "wv": f(np.asarray(wv, np.float32).T * WS),
            "bq": dup_b(bq, 1.0 / WS), "bk": dup_b(bk, WS),
            "gbv": f(gbv),
            "invg": f(np.full((P, 2 * P), WS / g, np.float32)),
        }

    c1 = common(wq1, bq1, wk2, bk2, wv2, bv2, gamma1)
    c2 = common(wq2, bq2, wk1, bk1, wv1, bv1, gamma2)
    maps = []
    for b in range(B):
        maps.append({"xq": f(x1f[b]), "xkv": f(x2f[b]), **c1})
    for b in range(B):
        maps.append({"xq": f(x2f[b]), "xkv": f(x1f[b]), **c2})
    return maps


def kernel(**inputs):
    nc = _build()
    in_maps = make_in_maps(**inputs)
    res = run_bass_kernel_spmd(nc, in_maps, list(range(8))).results
    out1 = np.stack([res[b]["out"].reshape(C, H, W) for b in range(B)])
    out2 = np.stack([res[B + b]["out"].reshape(C, H, W) for b in range(B)])
    return out1, out2
